# revision 12
# baseline (speedup 1.0000x reference)
"""BankModulatedConv Trainium2 kernel (v2: ic-split conv pipeline).

Problem (per sample b of B=8, one NeuronCore per sample):
  w = softmax(bank_request[b])                        # (16,)
  kern = sum_f w[f] * bank_weight[f]                  # (o, i, kh, kw) = (256, 256, 3, 3)
  kern *= (1 + style[b, i])                           # input-channel modulation
  kern *= rsqrt(sum_{i,kh,kw} kern^2 + 1e-8)          # per-o L2 demodulation
  y[b] = conv2d(x[b], kern, stride 1, SAME)           # (256, 64, 64)

v2 mapping (data-parallel over batch; all math on device):
  - The bank ships host-rearranged to 32 half-tiles
      [oc(2), ic(2), fh(8), i(128)] x [f2(2), o_local(128), khw(9)]  (bf16)
    issued in exact consumption order on the sync HWDGE queue with a
    shared 6-buffer pool tag, so arrival order tracks consumption and
    the stream self-paces: block (0,0) lands first, then x, then the
    remaining blocks back-to-back at aggregate HBM bandwidth.
  - Conv is split by input-channel half (ic): as soon as kern(oc0,ic0)
    is mixed (~1/4 of the bank stream), conv starts accumulating the
    ic0 taps of 4 spatial-tile PSUMs, leaving them open; the ic1 taps
    land when kern(oc0,ic1) is ready. This pulls ~17us of conv forward
    into what used to be DMA-starved mix time.
  - Style modulation is folded into the per-f mix weights
    (wsty[i,f] = softmax_w[f] * (1+style[i])), so mix PSUM->SBUF copies
    are plain copies and the DVE MAC chain needs no extra style pass.
  - Mix blocks (0,0), (0,1), (1,0) run on TensorE as 16 accumulated
    diag-weighted bf16 matmuls each; block (1,1) runs as a DVE
    scalar_tensor_tensor MAC chain so the PE can keep convolving.
  - demod: square + reduce-over-khw on DVE, cross-i reduction via a
    ones-vector matmul; rsqrt scale applied per output channel in the
    ScalarE conv-PSUM->y copy.
  - x ships host-pre-padded bf16; conv matmuls are bf16 (1 row/cycle,
    same as f32r, but half the DMA/SBUF traffic).
"""
import sys

if "/opt/trn_rl_repo" not in sys.path:
    sys.path.insert(0, "/opt/trn_rl_repo")

import numpy as np
import concourse.bacc as bacc
import concourse.mybir as mybir
import concourse.tile as tile
from concourse.alu_op_type import AluOpType
from concourse.bass_utils import run_bass_kernel_spmd

dt = mybir.dt
AF = mybir.ActivationFunctionType

B, F, D, KK, H, W = 8, 16, 256, 3, 64, 64
HW = H * W            # 4096
KHW = KK * KK         # 9
IC = D // 128         # 2 i-chunks
OC = D // 128         # 2 o-chunks
FH = 8                # f half-tiles per (oc, ic) block
FPT = 2               # f per half-tile
OCK = 128 * KHW       # 1152 free elems per (o_chunk, khw) group
TROW = FPT * OCK      # 2304 bf16 elems per DMA row
CONST = 257           # trailing constant columns on the first half-tile
PW = W + 2            # padded width 66
PH_ = H + 2           # padded height 66
NS = 8                # spatial tiles (8 rows each)
SROWS = H // NS       # 8 rows per spatial tile
SN = SROWS * W        # 512 = conv matmul moving size
G0 = (0, 1, 2, 3)     # first PSUM group of spatial tiles
G1 = (4, 5, 6, 7)

_COMPILED = None


def _build(num_devices=B):
    nc = bacc.Bacc("TRN2", target_bir_lowering=False, debug=False,
                   num_devices=num_devices)

    x_d = nc.dram_tensor("x", [D, PH_ * PW], dt.bfloat16, kind="ExternalInput").ap()
    bank_d = nc.dram_tensor("bank", [OC * IC * FH * 128, TROW + CONST], dt.bfloat16,
                            kind="ExternalInput").ap()
    breq_d = nc.dram_tensor("breq", [1, F], dt.float32, kind="ExternalInput").ap()
    sty_d = nc.dram_tensor("sty", [1, D], dt.float32, kind="ExternalInput").ap()
    y_d = nc.dram_tensor("y", [D, HW], dt.float32, kind="ExternalOutput").ap()

    f32, f32r, bf16 = dt.float32, dt.float32r, dt.bfloat16

    with tile.TileContext(nc) as tc:
        with (
            tc.tile_pool(name="setup", bufs=1) as setup,
            tc.tile_pool(name="xp", bufs=1) as xp,
            tc.tile_pool(name="bankp", bufs=4) as bankp,
            tc.tile_pool(name="kern", bufs=1) as kernp,
            tc.tile_pool(name="yout", bufs=4) as youtp,
            tc.tile_pool(name="mixps", bufs=1, space="PSUM") as mixps,
            tc.tile_pool(name="convps", bufs=4, space="PSUM") as convps,
            tc.tile_pool(name="normps", bufs=1, space="PSUM") as normps,
        ):
            # tiny control DMAs first so they land before the bank megabytes
            breq = setup.tile([1, F], dt.float32)
            nc.sync.dma_start(breq[:], breq_d[:])
            styrow = setup.tile([1, D], dt.float32)
            nc.sync.dma_start(styrow[:], sty_d[:])

            # ---------- bank + x DMAs, in exact consumption order ----------
            bts = {}

            def issue_block(oc, ic):
                for fh in range(FH):
                    wide = oc == 0 and ic == 0 and fh == 0
                    cols = TROW + CONST if wide else TROW
                    bt = bankp.tile([128, cols], bf16, tag="bank")
                    row0 = ((oc * IC + ic) * FH + fh) * 128
                    nc.sync.dma_start(bt[:], bank_d[row0:row0 + 128, 0:cols])
                    bts[(oc, ic, fh)] = bt

            issue_block(0, 0)
            # x ships as row-chunks so conv taps aren't gated on the whole-tile
            # DMA completing (chunk 0 covers G0's taps)
            XSPLIT = 36 * PW
            xpads = []
            for ic in range(IC):
                xpad = xp.tile([128, PH_ * PW], bf16, tag=f"xpad{ic}",
                               name=f"xpad{ic}")
                xpads.append(xpad)
            for ic, (lo, hi) in ((0, (0, XSPLIT)), (0, (XSPLIT, PH_ * PW)),
                                 (1, (0, XSPLIT)), (1, (XSPLIT, PH_ * PW))):
                nc.sync.dma_start(xpads[ic][:, lo:hi],
                                  x_d[ic * 128:(ic + 1) * 128, lo:hi])
            issue_block(0, 1)
            issue_block(1, 0)
            issue_block(1, 1)

            # ---------- setup: softmax weights, style, per-f diag tiles ----------
            bt0 = bts[(0, 0, 0)]
            ident = bt0[:, TROW:TROW + 128]            # I_128 (bf16-exact)
            onescol = bt0[:, TROW + 128:TROW + 129]
            onesrow_b = bt0[0:1, TROW + 129:TROW + 257]
            ones11_b = bt0[0:1, TROW + 129:TROW + 130]

            # softmax without the max-shift: inputs are O(1) so exp can't
            # overflow, and the shift is dead weight on the critical path
            ex = setup.tile([1, F], f32)
            nc.scalar.activation(ex[:], breq[:], AF.Exp, bias=0.0, scale=1.0)
            sm = setup.tile([1, 1], f32)
            nc.vector.reduce_sum(sm[:], ex[:], axis=mybir.AxisListType.X)
            rs = setup.tile([1, 1], f32)
            nc.vector.reciprocal(rs[:], sm[:])
            wrow = setup.tile([1, F], f32)
            nc.vector.tensor_scalar(out=wrow[:], in0=ex[:], scalar1=rs[:],
                                    scalar2=None, op0=AluOpType.mult)
            # broadcast w across partitions with a K=1 bf16 matmul
            wrow_b = setup.tile([1, F], bf16)
            with nc.allow_low_precision(reason="broadcast only"):
                nc.vector.tensor_copy(wrow_b[:], wrow[:])
            wbps = normps.tile([128, F], f32, tag="norm")
            nc.tensor.matmul(wbps[:], onesrow_b[:], wrow_b[:], start=True, stop=True)
            wbc = setup.tile([128, F], f32)
            nc.vector.tensor_copy(wbc[:], wbps[:])

            # (1 + style) spread to per-partition columns via K=1 matmuls
            sty1 = setup.tile([1, D], f32)
            nc.scalar.activation(sty1[:], styrow[:], AF.Copy, bias=1.0, scale=1.0)
            sty1b = setup.tile([1, D], bf16)
            with nc.allow_low_precision(reason="style factors; bf16 matches bank"):
                nc.vector.tensor_copy(sty1b[:], sty1[:])
            styps = normps.tile([128, IC], f32, tag="norm")
            stycols = []
            for ic in range(IC):
                nc.tensor.matmul(styps[:, ic:ic + 1],
                                 sty1b[0:1, ic * 128:(ic + 1) * 128],
                                 ones11_b, start=True, stop=True)
                sc = setup.tile([128, 1], f32, tag=f"sty{ic}")
                nc.scalar.activation(sc[:], styps[:, ic:ic + 1], AF.Copy,
                                     bias=0.0, scale=1.0)
                stycols.append(sc)

            # wsty[i, f] = w[f] * (1 + style[i]) -- style folded into mix weights
            wstys = []
            for ic in range(IC):
                ws = setup.tile([128, F], f32, tag=f"wsty{ic}")
                nc.vector.tensor_scalar(out=ws[:], in0=wbc[:],
                                        scalar1=stycols[ic][:],
                                        scalar2=None, op0=AluOpType.mult)
                wstys.append(ws)

            # copy the constants out of bt0 first: its pool slot must free as
            # soon as mix reads f0/f1, or the gated fh6/fh7/x dma triggers
            # (in-order sync queue) stall the x stream into conv's window
            idcp = setup.tile([128, 128], bf16)
            nc.vector.tensor_copy(idcp[:], ident)
            ones_r = setup.tile([128, 1], f32r)
            nc.vector.tensor_copy(ones_r[:], onescol)
            ones12 = setup.tile([1, 2], f32)
            nc.vector.memset(ones12[:], 1.0)

            # per-(ic, f) diagonal lhsT tiles diag(wsty), bf16 for the mix matmuls
            diags = {}
            with nc.allow_low_precision(reason="bf16 diag weights; mix accumulates f32"):
                for ic in range(IC):
                    for f in range(F):
                        dg = setup.tile([128, 128], bf16, tag=f"dg{ic}_{f}")
                        nc.vector.tensor_scalar(out=dg[:], in0=idcp[:],
                                                scalar1=wstys[ic][:, f:f + 1],
                                                scalar2=None, op0=AluOpType.mult)
                        diags[(ic, f)] = dg

            # ---------- mix / demod / norm helpers ----------
            SL = ((0, 512), (512, 1024), (1024, OCK))
            km = {}
            normcols = {}

            def mix_pe(oc, ic, copy_engine):
                kt = kernp.tile([128, OCK], bf16, tag=f"kern{oc}{ic}", name=f"kt{oc}{ic}")
                ps0 = mixps.tile([128, 512], f32, tag="mix0", name=f"m0{oc}{ic}")
                ps1 = mixps.tile([128, 512], f32, tag="mix1", name=f"m1{oc}{ic}")
                ps2 = mixps.tile([128, OCK - 1024], f32, tag="mix2", name=f"m2{oc}{ic}")
                pss = (ps0, ps1, ps2)
                for f in range(F):
                    bt = bts[(oc, ic, f // FPT)]
                    fo = (f % FPT) * OCK
                    for (lo, hi), ps in zip(SL, pss):
                        nc.tensor.matmul(ps[:], diags[(ic, f)][:],
                                         bt[:, fo + lo:fo + hi],
                                         start=(f == 0), stop=(f == F - 1))
                # copies chase the per-slice accumulation ends
                with nc.allow_low_precision(reason="kern stored bf16 for conv"):
                    for si, ((lo, hi), ps) in enumerate(zip(SL, pss)):
                        eng = copy_engine
                        if eng == "both":
                            eng = "scalar" if si == 1 else "vector"
                        if eng == "vector":
                            nc.vector.tensor_copy(kt[:, lo:hi], ps[:])
                        else:
                            nc.scalar.activation(kt[:, lo:hi], ps[:], AF.Copy,
                                                 bias=0.0, scale=1.0)
                km[(ic, oc)] = kt

            def mix_mac(oc, ic, mid_hook=None):
                # DVE scalar_tensor_tensor MAC chain; style already in wsty
                kt = kernp.tile([128, OCK], bf16, tag=f"kern{oc}{ic}", name=f"kt{oc}{ic}")
                acc0 = kernp.tile([128, OCK], f32, tag="macacc0", name="macacc0")
                acc1 = kernp.tile([128, OCK], f32, tag="macacc1", name="macacc1")
                accs = (acc0, acc1)
                ws = wstys[ic]
                with nc.allow_low_precision(reason="bf16 in, f32 acc, bf16 out"):
                    nc.vector.tensor_scalar(
                        out=accs[0][:], in0=bts[(oc, ic, 0)][:, 0:OCK],
                        scalar1=ws[:, 0:1], scalar2=None, op0=AluOpType.mult)
                    for f in range(1, F):
                        bt = bts[(oc, ic, f // FPT)]
                        fo = (f % FPT) * OCK
                        out = kt if f == F - 1 else accs[f % 2]
                        nc.vector.scalar_tensor_tensor(
                            out=out[:], in0=bt[:, fo:fo + OCK],
                            scalar=ws[:, f:f + 1], in1=accs[(f + 1) % 2][:],
                            op0=AluOpType.mult, op1=AluOpType.add)
                        if mid_hook is not None and f == mid_hook[0]:
                            mid_hook[1]()
                km[(ic, oc)] = kt

            def demod_dve(oc, ic):
                kt = km[(ic, oc)]
                scr = kernp.tile([128, OCK], f32r, tag="sqscratch", name=f"scr{oc}{ic}")
                nc.vector.tensor_mul(scr[:], kt[:], kt[:])
                redk = kernp.tile([128, 128], f32r, tag=f"redk{oc}{ic}",
                                  name=f"redk{oc}{ic}")
                with nc.allow_low_precision(reason="f32r is 4-byte"):
                    nc.vector.tensor_reduce(
                        redk[:], scr[:, :].rearrange("p (o r) -> p o r", r=KHW),
                        axis=mybir.AxisListType.X, op=AluOpType.add)
                return redk

            def demod_pe(npsum, redk, first, last):
                nc.tensor.matmul(npsum[:], ones_r[:], redk[:], start=first, stop=last)

            def norm_final(oc, npsum):
                nrow = setup.tile([1, 128], f32, tag=f"nrow{oc}", name=f"nrow{oc}")
                nc.vector.tensor_scalar_add(nrow[:], npsum[:], 1e-8)
                nsq = setup.tile([1, 128], f32, tag=f"nsq{oc}", name=f"nsq{oc}")
                nc.scalar.activation(nsq[:], nrow[:], AF.Sqrt, bias=0.0, scale=1.0)
                nrec = setup.tile([1, 128], f32, tag=f"nrec{oc}", name=f"nrec{oc}")
                nc.vector.reciprocal(nrec[:], nsq[:])
                ntr = normps.tile([128, 2], f32, tag="norm", name=f"ntr{oc}")
                nc.tensor.matmul(ntr[:], nrec[:], ones12[:], start=True, stop=True)
                ncol = setup.tile([128, 1], f32, tag=f"ncol{oc}", name=f"ncol{oc}")
                nc.scalar.activation(ncol[:], ntr[:, 0:1], AF.Copy, bias=0.0, scale=1.0)
                normcols[oc] = ncol

            # ---------- conv sweeps ----------
            cps = {}
            started = set()

            def conv_close(oc, s):
                yt = youtp.tile([128, SN], f32, tag="y", name=f"y{oc}{s}")
                nc.scalar.activation(yt[:], cps[(oc, s)][:], AF.Copy,
                                     bias=0.0, scale=normcols[oc][:])
                r0 = s * SROWS
                nc.scalar.dma_start(
                    y_d[oc * 128:(oc + 1) * 128, r0 * W:(r0 + SROWS) * W], yt[:])
                del cps[(oc, s)]

            def conv_mm(oc, s, ic, kh, kw, last):
                if (oc, s) not in cps:
                    cps[(oc, s)] = convps.tile([128, SN], f32, tag="conv",
                                               name=f"c{oc}{s}")
                r0 = s * SROWS
                xv = xpads[ic][:, :].rearrange("p (r c) -> p r c", c=PW)
                kv = km[(ic, oc)][:, :].rearrange("p (o r) -> p o r", r=KHW)
                nc.tensor.matmul(
                    cps[(oc, s)][:], kv[:, :, kh * KK + kw],
                    xv[:, r0 + kh:r0 + kh + SROWS, kw:kw + W],
                    start=(oc, s) not in started, stop=last)
                started.add((oc, s))

            def conv_tapmajor(oc, tiles, ic):
                # open sweep: tap-major (lhsT reused across tiles), no closes
                for kh in range(KK):
                    for kw in range(KK):
                        for s in tiles:
                            conv_mm(oc, s, ic, kh, kw, last=False)

            def conv_tilemajor(oc, tiles, ics):
                # closing sweep: tile-major so PSUM banks free progressively
                for s in tiles:
                    for ici, ic in enumerate(ics):
                        for kh in range(KK):
                            for kw in range(KK):
                                conv_mm(oc, s, ic, kh, kw,
                                        last=(ici == len(ics) - 1 and
                                              kh == KK - 1 and kw == KK - 1))
                    conv_close(oc, s)

            # ---------- schedule ----------
            # PE: mix00 -> G0(oc0) ic0 taps -> mix01 -> norm0 mms -> mix10
            #     -> G0(oc0) ic1 taps (close) -> G1(oc0) full (close)
            #     -> G0(oc1) ic0 taps -> norm1 mms -> G0(oc1) ic1 (close)
            #     -> G1(oc1) full (close)
            # DVE: diags -> kern copies/squares (0,*) -> norm0 tail
            #     -> MAC chain (1,1) with squares(1,0) spliced in -> norm1 tail
            mix_pe(0, 0, copy_engine="both")
            redk00 = demod_dve(0, 0)
            conv_tapmajor(0, G0, 0)
            npsum0 = normps.tile([1, 128], f32, tag="norm", name="npsum0")
            demod_pe(npsum0, redk00, True, False)
            mix_pe(0, 1, copy_engine="vector")
            redk01 = demod_dve(0, 1)

            mix_pe(1, 0, copy_engine="scalar")
            demod_pe(npsum0, redk01, False, True)
            norm_final(0, npsum0)

            # kern(1,1) mixed on DVE while PE convolves; demod(1,0)
            # squares are spliced into the chain once kern(1,0) lands
            redkbox = {}

            def sq10():
                redkbox["r10"] = demod_dve(1, 0)

            mix_mac(1, 1, mid_hook=(8, sq10))
            redk11 = demod_dve(1, 1)

            conv_tilemajor(0, G0, (1,))
            conv_tilemajor(0, G1, (0, 1))

            conv_tapmajor(1, G0, 0)
            npsum1 = normps.tile([1, 128], f32, tag="norm", name="npsum1")
            demod_pe(npsum1, redkbox["r10"], True, False)
            demod_pe(npsum1, redk11, False, True)
            norm_final(1, npsum1)
            conv_tilemajor(1, G0, (1,))
            conv_tilemajor(1, (4, 5, 6), (0, 1))

            # last spatial tile as two 4-row halves: the first half's y copy
            # and DMA overlap the second half's matmuls, trimming the tail
            for half in range(2):
                cp = convps.tile([128, SN // 2], f32, tag="conv",
                                 name=f"c17h{half}")
                r0 = 7 * SROWS + 4 * half
                for ici, ic in enumerate((0, 1)):
                    xv = xpads[ic][:, :].rearrange("p (r c) -> p r c", c=PW)
                    kv = km[(ic, 1)][:, :].rearrange("p (o r) -> p o r", r=KHW)
                    for kh in range(KK):
                        for kw in range(KK):
                            nc.tensor.matmul(
                                cp[:], kv[:, :, kh * KK + kw],
                                xv[:, r0 + kh:r0 + kh + 4, kw:kw + W],
                                start=(ici == 0 and kh == 0 and kw == 0),
                                stop=(ici == 1 and kh == KK - 1 and kw == KK - 1))
                yt = youtp.tile([128, SN // 2], f32, tag="y", name=f"y17h{half}")
                nc.scalar.activation(yt[:], cp[:], AF.Copy,
                                     bias=0.0, scale=normcols[1][:])
                nc.scalar.dma_start(
                    y_d[128:256, r0 * W:(r0 + 4) * W], yt[:])

    nc.compile()
    return nc


def _get_compiled():
    global _COMPILED
    if _COMPILED is None:
        _COMPILED = _build()
    return _COMPILED


def _make_in_maps(x, bank_request, style, bank_weight):
    # bank: (F, O, I, KH, KW) -> rows [oc, ic, fh, i], cols [f2, o_local, khw], bf16
    bf16_np = mybir.dt.np(mybir.dt.bfloat16)
    A = bank_weight.astype(np.float32).reshape(FH, FPT, OC, 128, IC, 128, KHW)
    #                     dims: (fh, f2, oc, o_local, ic, i, khw)
    core = A.transpose(2, 4, 0, 5, 1, 3, 6).reshape(OC * IC * FH * 128, TROW)
    bankT = np.zeros((OC * IC * FH * 128, TROW + CONST), dtype=np.float32)
    bankT[:, 0:TROW] = core
    bankT[0:128, TROW:TROW + 128] = np.eye(128, dtype=np.float32)
    bankT[0:128, TROW + 128] = 1.0
    bankT[0, TROW + 129:TROW + 257] = 1.0
    bankT = np.ascontiguousarray(bankT).astype(bf16_np)
    maps = []
    xpad = np.zeros((B, D, PH_, PW), dtype=np.float32)
    xpad[:, :, 1:1 + H, 1:1 + W] = x.astype(np.float32).reshape(B, D, H, W)
    xpad = xpad.astype(bf16_np)
    for b in range(B):
        maps.append({
            "x": np.ascontiguousarray(xpad[b].reshape(D, PH_ * PW)),
            "bank": bankT,
            "breq": np.ascontiguousarray(
                bank_request[b].astype(np.float32).reshape(1, F)),
            "sty": np.ascontiguousarray(style[b].astype(np.float32).reshape(1, D)),
        })
    return maps


def run(inputs, trace=False, **trace_kwargs):
    nc = _get_compiled()
    in_maps = _make_in_maps(inputs["x"], inputs["bank_request"],
                            inputs["style"], inputs["bank_weight"])
    # The first execution of a freshly compiled NEFF occasionally dies with
    # NRT_EXEC_UNIT_UNRECOVERABLE on this runtime; a plain retry succeeds.
    last_exc = None
    for _ in range(3):
        try:
            res = run_bass_kernel_spmd(nc, in_maps, core_ids=list(range(B)),
                                       trace=trace, **trace_kwargs)
            y = np.stack([res.results[b]["y"].reshape(D, H, W) for b in range(B)],
                         axis=0)
            return y, res
        except Exception as e:  # noqa: BLE001
            last_exc = e
    raise last_exc


def kernel(x, bank_request, style, bank_weight):
    y, _ = run({"x": np.asarray(x), "bank_request": np.asarray(bank_request),
                "style": np.asarray(style), "bank_weight": np.asarray(bank_weight)})
    return y


# revision 13
# speedup vs baseline: 1.0062x; 1.0062x over previous
"""BankModulatedConv Trainium2 kernel (v2: ic-split conv pipeline).

Problem (per sample b of B=8, one NeuronCore per sample):
  w = softmax(bank_request[b])                        # (16,)
  kern = sum_f w[f] * bank_weight[f]                  # (o, i, kh, kw) = (256, 256, 3, 3)
  kern *= (1 + style[b, i])                           # input-channel modulation
  kern *= rsqrt(sum_{i,kh,kw} kern^2 + 1e-8)          # per-o L2 demodulation
  y[b] = conv2d(x[b], kern, stride 1, SAME)           # (256, 64, 64)

v2 mapping (data-parallel over batch; all math on device):
  - The bank ships host-rearranged to 32 half-tiles
      [oc(2), ic(2), fh(8), i(128)] x [f2(2), o_local(128), khw(9)]  (bf16)
    issued in exact consumption order on the sync HWDGE queue with a
    shared 6-buffer pool tag, so arrival order tracks consumption and
    the stream self-paces: block (0,0) lands first, then x, then the
    remaining blocks back-to-back at aggregate HBM bandwidth.
  - Conv is split by input-channel half (ic): as soon as kern(oc0,ic0)
    is mixed (~1/4 of the bank stream), conv starts accumulating the
    ic0 taps of 4 spatial-tile PSUMs, leaving them open; the ic1 taps
    land when kern(oc0,ic1) is ready. This pulls ~17us of conv forward
    into what used to be DMA-starved mix time.
  - Style modulation is folded into the per-f mix weights
    (wsty[i,f] = softmax_w[f] * (1+style[i])), so mix PSUM->SBUF copies
    are plain copies and the DVE MAC chain needs no extra style pass.
  - Mix blocks (0,0), (0,1), (1,0) run on TensorE as 16 accumulated
    diag-weighted bf16 matmuls each; block (1,1) runs as a DVE
    scalar_tensor_tensor MAC chain so the PE can keep convolving.
  - demod: square + reduce-over-khw on DVE, cross-i reduction via a
    ones-vector matmul; rsqrt scale applied per output channel in the
    ScalarE conv-PSUM->y copy.
  - x ships host-pre-padded bf16; conv matmuls are bf16 (1 row/cycle,
    same as f32r, but half the DMA/SBUF traffic).
"""
import sys

if "/opt/trn_rl_repo" not in sys.path:
    sys.path.insert(0, "/opt/trn_rl_repo")

import numpy as np
import concourse.bacc as bacc
import concourse.mybir as mybir
import concourse.tile as tile
from concourse.alu_op_type import AluOpType
from concourse.bass_utils import run_bass_kernel_spmd

dt = mybir.dt
AF = mybir.ActivationFunctionType

B, F, D, KK, H, W = 8, 16, 256, 3, 64, 64
HW = H * W            # 4096
KHW = KK * KK         # 9
IC = D // 128         # 2 i-chunks
OC = D // 128         # 2 o-chunks
FH = 8                # f half-tiles per (oc, ic) block
FPT = 2               # f per half-tile
OCK = 128 * KHW       # 1152 free elems per (o_chunk, khw) group
TROW = FPT * OCK      # 2304 bf16 elems per DMA row
CONST = 257           # trailing constant columns on the first half-tile
PW = W + 2            # padded width 66
PH_ = H + 2           # padded height 66
NS = 8                # spatial tiles (8 rows each)
SROWS = H // NS       # 8 rows per spatial tile
SN = SROWS * W        # 512 = conv matmul moving size
G0 = (0, 1, 2, 3)     # first PSUM group of spatial tiles
G1 = (4, 5, 6, 7)

_COMPILED = None


def _build(num_devices=B):
    nc = bacc.Bacc("TRN2", target_bir_lowering=False, debug=False,
                   num_devices=num_devices)

    x_d = nc.dram_tensor("x", [D, PH_ * PW], dt.bfloat16, kind="ExternalInput").ap()
    bank_d = nc.dram_tensor("bank", [OC * IC * FH * 128, TROW + CONST], dt.bfloat16,
                            kind="ExternalInput").ap()
    breq_d = nc.dram_tensor("breq", [1, F], dt.float32, kind="ExternalInput").ap()
    sty_d = nc.dram_tensor("sty", [1, D], dt.float32, kind="ExternalInput").ap()
    y_d = nc.dram_tensor("y", [D, HW], dt.float32, kind="ExternalOutput").ap()

    f32, f32r, bf16 = dt.float32, dt.float32r, dt.bfloat16

    with tile.TileContext(nc) as tc:
        with (
            tc.tile_pool(name="setup", bufs=1) as setup,
            tc.tile_pool(name="xp", bufs=1) as xp,
            tc.tile_pool(name="bankp", bufs=6) as bankp,
            tc.tile_pool(name="kern", bufs=1) as kernp,
            tc.tile_pool(name="yout", bufs=4) as youtp,
            tc.tile_pool(name="mixps", bufs=1, space="PSUM") as mixps,
            tc.tile_pool(name="convps", bufs=4, space="PSUM") as convps,
            tc.tile_pool(name="normps", bufs=1, space="PSUM") as normps,
        ):
            # tiny control DMAs first so they land before the bank megabytes
            breq = setup.tile([1, F], dt.float32)
            nc.sync.dma_start(breq[:], breq_d[:])
            styrow = setup.tile([1, D], dt.float32)
            nc.sync.dma_start(styrow[:], sty_d[:])

            # ---------- bank + x DMAs, in exact consumption order ----------
            bts = {}

            def issue_block(oc, ic):
                for fh in range(FH):
                    wide = oc == 0 and ic == 0 and fh == 0
                    cols = TROW + CONST if wide else TROW
                    bt = bankp.tile([128, cols], bf16, tag="bank")
                    row0 = ((oc * IC + ic) * FH + fh) * 128
                    nc.sync.dma_start(bt[:], bank_d[row0:row0 + 128, 0:cols])
                    bts[(oc, ic, fh)] = bt

            issue_block(0, 0)
            # x ships as row-chunks so conv taps aren't gated on the whole-tile
            # DMA completing (chunk 0 covers G0's taps)
            XSPLIT = 36 * PW
            xpads = []
            for ic in range(IC):
                xpad = xp.tile([128, PH_ * PW], bf16, tag=f"xpad{ic}",
                               name=f"xpad{ic}")
                xpads.append(xpad)
            for ic, (lo, hi) in ((0, (0, XSPLIT)), (0, (XSPLIT, PH_ * PW)),
                                 (1, (0, XSPLIT)), (1, (XSPLIT, PH_ * PW))):
                nc.sync.dma_start(xpads[ic][:, lo:hi],
                                  x_d[ic * 128:(ic + 1) * 128, lo:hi])
            issue_block(0, 1)
            issue_block(1, 0)
            issue_block(1, 1)

            # ---------- setup: softmax weights, style, per-f diag tiles ----------
            bt0 = bts[(0, 0, 0)]
            ident = bt0[:, TROW:TROW + 128]            # I_128 (bf16-exact)
            onescol = bt0[:, TROW + 128:TROW + 129]
            onesrow_b = bt0[0:1, TROW + 129:TROW + 257]
            ones11_b = bt0[0:1, TROW + 129:TROW + 130]

            # softmax without the max-shift: inputs are O(1) so exp can't
            # overflow, and the shift is dead weight on the critical path
            ex = setup.tile([1, F], f32)
            nc.scalar.activation(ex[:], breq[:], AF.Exp, bias=0.0, scale=1.0)
            sm = setup.tile([1, 1], f32)
            nc.vector.reduce_sum(sm[:], ex[:], axis=mybir.AxisListType.X)
            rs = setup.tile([1, 1], f32)
            nc.vector.reciprocal(rs[:], sm[:])
            wrow = setup.tile([1, F], f32)
            nc.vector.tensor_scalar(out=wrow[:], in0=ex[:], scalar1=rs[:],
                                    scalar2=None, op0=AluOpType.mult)
            # broadcast w across partitions with a K=1 bf16 matmul
            wrow_b = setup.tile([1, F], bf16)
            with nc.allow_low_precision(reason="broadcast only"):
                nc.vector.tensor_copy(wrow_b[:], wrow[:])
            wbps = normps.tile([128, F], f32, tag="norm")
            nc.tensor.matmul(wbps[:], onesrow_b[:], wrow_b[:], start=True, stop=True)
            wbc = setup.tile([128, F], f32)
            nc.vector.tensor_copy(wbc[:], wbps[:])

            # (1 + style) spread to per-partition columns via K=1 matmuls
            sty1 = setup.tile([1, D], f32)
            nc.scalar.activation(sty1[:], styrow[:], AF.Copy, bias=1.0, scale=1.0)
            sty1b = setup.tile([1, D], bf16)
            with nc.allow_low_precision(reason="style factors; bf16 matches bank"):
                nc.vector.tensor_copy(sty1b[:], sty1[:])
            styps = normps.tile([128, IC], f32, tag="norm")
            stycols = []
            for ic in range(IC):
                nc.tensor.matmul(styps[:, ic:ic + 1],
                                 sty1b[0:1, ic * 128:(ic + 1) * 128],
                                 ones11_b, start=True, stop=True)
                sc = setup.tile([128, 1], f32, tag=f"sty{ic}")
                nc.scalar.activation(sc[:], styps[:, ic:ic + 1], AF.Copy,
                                     bias=0.0, scale=1.0)
                stycols.append(sc)

            # wsty[i, f] = w[f] * (1 + style[i]) -- style folded into mix weights
            wstys = []
            for ic in range(IC):
                ws = setup.tile([128, F], f32, tag=f"wsty{ic}")
                nc.vector.tensor_scalar(out=ws[:], in0=wbc[:],
                                        scalar1=stycols[ic][:],
                                        scalar2=None, op0=AluOpType.mult)
                wstys.append(ws)

            # copy the constants out of bt0 first: its pool slot must free as
            # soon as mix reads f0/f1, or the gated fh6/fh7/x dma triggers
            # (in-order sync queue) stall the x stream into conv's window
            idcp = setup.tile([128, 128], bf16)
            nc.vector.tensor_copy(idcp[:], ident)
            ones_r = setup.tile([128, 1], f32r)
            nc.vector.tensor_copy(ones_r[:], onescol)
            ones12 = setup.tile([1, 2], f32)
            nc.vector.memset(ones12[:], 1.0)

            # per-(ic, f) diagonal lhsT tiles diag(wsty), bf16 for the mix matmuls
            diags = {}
            with nc.allow_low_precision(reason="bf16 diag weights; mix accumulates f32"):
                for ic in range(IC):
                    for f in range(F):
                        dg = setup.tile([128, 128], bf16, tag=f"dg{ic}_{f}")
                        nc.vector.tensor_scalar(out=dg[:], in0=idcp[:],
                                                scalar1=wstys[ic][:, f:f + 1],
                                                scalar2=None, op0=AluOpType.mult)
                        diags[(ic, f)] = dg

            # ---------- mix / demod / norm helpers ----------
            SL = ((0, 512), (512, 1024), (1024, OCK))
            km = {}
            normcols = {}

            def mix_pe(oc, ic, copy_engine):
                kt = kernp.tile([128, OCK], bf16, tag=f"kern{oc}{ic}", name=f"kt{oc}{ic}")
                ps0 = mixps.tile([128, 512], f32, tag="mix0", name=f"m0{oc}{ic}")
                ps1 = mixps.tile([128, 512], f32, tag="mix1", name=f"m1{oc}{ic}")
                ps2 = mixps.tile([128, OCK - 1024], f32, tag="mix2", name=f"m2{oc}{ic}")
                pss = (ps0, ps1, ps2)
                for f in range(F):
                    bt = bts[(oc, ic, f // FPT)]
                    fo = (f % FPT) * OCK
                    for (lo, hi), ps in zip(SL, pss):
                        nc.tensor.matmul(ps[:], diags[(ic, f)][:],
                                         bt[:, fo + lo:fo + hi],
                                         start=(f == 0), stop=(f == F - 1))
                # copies chase the per-slice accumulation ends
                with nc.allow_low_precision(reason="kern stored bf16 for conv"):
                    for si, ((lo, hi), ps) in enumerate(zip(SL, pss)):
                        eng = copy_engine
                        if eng == "both":
                            eng = "scalar" if si == 1 else "vector"
                        if eng == "vector":
                            nc.vector.tensor_copy(kt[:, lo:hi], ps[:])
                        else:
                            nc.scalar.activation(kt[:, lo:hi], ps[:], AF.Copy,
                                                 bias=0.0, scale=1.0)
                km[(ic, oc)] = kt

            def mix_mac(oc, ic, mid_hook=None):
                # DVE scalar_tensor_tensor MAC chain; style already in wsty
                kt = kernp.tile([128, OCK], bf16, tag=f"kern{oc}{ic}", name=f"kt{oc}{ic}")
                acc0 = kernp.tile([128, OCK], f32, tag="macacc0", name="macacc0")
                acc1 = kernp.tile([128, OCK], f32, tag="macacc1", name="macacc1")
                accs = (acc0, acc1)
                ws = wstys[ic]
                with nc.allow_low_precision(reason="bf16 in, f32 acc, bf16 out"):
                    nc.vector.tensor_scalar(
                        out=accs[0][:], in0=bts[(oc, ic, 0)][:, 0:OCK],
                        scalar1=ws[:, 0:1], scalar2=None, op0=AluOpType.mult)
                    for f in range(1, F):
                        bt = bts[(oc, ic, f // FPT)]
                        fo = (f % FPT) * OCK
                        out = kt if f == F - 1 else accs[f % 2]
                        nc.vector.scalar_tensor_tensor(
                            out=out[:], in0=bt[:, fo:fo + OCK],
                            scalar=ws[:, f:f + 1], in1=accs[(f + 1) % 2][:],
                            op0=AluOpType.mult, op1=AluOpType.add)
                        if mid_hook is not None and f == mid_hook[0]:
                            mid_hook[1]()
                km[(ic, oc)] = kt

            def demod_dve(oc, ic):
                kt = km[(ic, oc)]
                scr = kernp.tile([128, OCK], f32r, tag="sqscratch", name=f"scr{oc}{ic}")
                nc.vector.tensor_mul(scr[:], kt[:], kt[:])
                redk = kernp.tile([128, 128], f32r, tag=f"redk{oc}{ic}",
                                  name=f"redk{oc}{ic}")
                with nc.allow_low_precision(reason="f32r is 4-byte"):
                    nc.vector.tensor_reduce(
                        redk[:], scr[:, :].rearrange("p (o r) -> p o r", r=KHW),
                        axis=mybir.AxisListType.X, op=AluOpType.add)
                return redk

            def demod_pe(npsum, redk, first, last):
                nc.tensor.matmul(npsum[:], ones_r[:], redk[:], start=first, stop=last)

            def norm_final(oc, npsum):
                nrow = setup.tile([1, 128], f32, tag=f"nrow{oc}", name=f"nrow{oc}")
                nc.vector.tensor_scalar_add(nrow[:], npsum[:], 1e-8)
                nsq = setup.tile([1, 128], f32, tag=f"nsq{oc}", name=f"nsq{oc}")
                nc.scalar.activation(nsq[:], nrow[:], AF.Sqrt, bias=0.0, scale=1.0)
                nrec = setup.tile([1, 128], f32, tag=f"nrec{oc}", name=f"nrec{oc}")
                nc.vector.reciprocal(nrec[:], nsq[:])
                ntr = normps.tile([128, 2], f32, tag="norm", name=f"ntr{oc}")
                nc.tensor.matmul(ntr[:], nrec[:], ones12[:], start=True, stop=True)
                ncol = setup.tile([128, 1], f32, tag=f"ncol{oc}", name=f"ncol{oc}")
                nc.scalar.activation(ncol[:], ntr[:, 0:1], AF.Copy, bias=0.0, scale=1.0)
                normcols[oc] = ncol

            # ---------- conv sweeps ----------
            cps = {}
            started = set()

            def conv_close(oc, s):
                yt = youtp.tile([128, SN], f32, tag="y", name=f"y{oc}{s}")
                nc.scalar.activation(yt[:], cps[(oc, s)][:], AF.Copy,
                                     bias=0.0, scale=normcols[oc][:])
                r0 = s * SROWS
                nc.scalar.dma_start(
                    y_d[oc * 128:(oc + 1) * 128, r0 * W:(r0 + SROWS) * W], yt[:])
                del cps[(oc, s)]

            def conv_mm(oc, s, ic, kh, kw, last):
                if (oc, s) not in cps:
                    cps[(oc, s)] = convps.tile([128, SN], f32, tag="conv",
                                               name=f"c{oc}{s}")
                r0 = s * SROWS
                xv = xpads[ic][:, :].rearrange("p (r c) -> p r c", c=PW)
                kv = km[(ic, oc)][:, :].rearrange("p (o r) -> p o r", r=KHW)
                nc.tensor.matmul(
                    cps[(oc, s)][:], kv[:, :, kh * KK + kw],
                    xv[:, r0 + kh:r0 + kh + SROWS, kw:kw + W],
                    start=(oc, s) not in started, stop=last)
                started.add((oc, s))

            def conv_tapmajor(oc, tiles, ic):
                # open sweep: tap-major (lhsT reused across tiles), no closes
                for kh in range(KK):
                    for kw in range(KK):
                        for s in tiles:
                            conv_mm(oc, s, ic, kh, kw, last=False)

            def conv_tilemajor(oc, tiles, ics):
                # closing sweep: tile-major so PSUM banks free progressively
                for s in tiles:
                    for ici, ic in enumerate(ics):
                        for kh in range(KK):
                            for kw in range(KK):
                                conv_mm(oc, s, ic, kh, kw,
                                        last=(ici == len(ics) - 1 and
                                              kh == KK - 1 and kw == KK - 1))
                    conv_close(oc, s)

            # ---------- schedule ----------
            # PE: mix00 -> G0(oc0) ic0 taps -> mix01 -> norm0 mms -> mix10
            #     -> G0(oc0) ic1 taps (close) -> G1(oc0) full (close)
            #     -> G0(oc1) ic0 taps -> norm1 mms -> G0(oc1) ic1 (close)
            #     -> G1(oc1) full (close)
            # DVE: diags -> kern copies/squares (0,*) -> norm0 tail
            #     -> MAC chain (1,1) with squares(1,0) spliced in -> norm1 tail
            mix_pe(0, 0, copy_engine="both")
            redk00 = demod_dve(0, 0)
            conv_tapmajor(0, G0, 0)
            npsum0 = normps.tile([1, 128], f32, tag="norm", name="npsum0")
            demod_pe(npsum0, redk00, True, False)
            mix_pe(0, 1, copy_engine="vector")
            redk01 = demod_dve(0, 1)

            mix_pe(1, 0, copy_engine="scalar")
            demod_pe(npsum0, redk01, False, True)
            norm_final(0, npsum0)

            # kern(1,1) mixed on DVE while PE convolves; demod(1,0)
            # squares are spliced into the chain once kern(1,0) lands
            redkbox = {}

            def sq10():
                redkbox["r10"] = demod_dve(1, 0)

            mix_mac(1, 1, mid_hook=(8, sq10))
            redk11 = demod_dve(1, 1)

            conv_tilemajor(0, G0, (1,))
            conv_tilemajor(0, G1, (0, 1))

            conv_tapmajor(1, G0, 0)
            npsum1 = normps.tile([1, 128], f32, tag="norm", name="npsum1")
            demod_pe(npsum1, redkbox["r10"], True, False)
            demod_pe(npsum1, redk11, False, True)
            norm_final(1, npsum1)
            conv_tilemajor(1, G0, (1,))
            conv_tilemajor(1, (4, 5, 6), (0, 1))

            # last spatial tile as two 4-row halves: the first half's y copy
            # and DMA overlap the second half's matmuls, trimming the tail
            for half in range(2):
                cp = convps.tile([128, SN // 2], f32, tag="conv",
                                 name=f"c17h{half}")
                r0 = 7 * SROWS + 4 * half
                for ici, ic in enumerate((0, 1)):
                    xv = xpads[ic][:, :].rearrange("p (r c) -> p r c", c=PW)
                    kv = km[(ic, 1)][:, :].rearrange("p (o r) -> p o r", r=KHW)
                    for kh in range(KK):
                        for kw in range(KK):
                            nc.tensor.matmul(
                                cp[:], kv[:, :, kh * KK + kw],
                                xv[:, r0 + kh:r0 + kh + 4, kw:kw + W],
                                start=(ici == 0 and kh == 0 and kw == 0),
                                stop=(ici == 1 and kh == KK - 1 and kw == KK - 1))
                yt = youtp.tile([128, SN // 2], f32, tag="y", name=f"y17h{half}")
                nc.scalar.activation(yt[:], cp[:], AF.Copy,
                                     bias=0.0, scale=normcols[1][:])
                nc.scalar.dma_start(
                    y_d[128:256, r0 * W:(r0 + 4) * W], yt[:])

    nc.compile()
    return nc


def _get_compiled():
    global _COMPILED
    if _COMPILED is None:
        _COMPILED = _build()
    return _COMPILED


def _make_in_maps(x, bank_request, style, bank_weight):
    # bank: (F, O, I, KH, KW) -> rows [oc, ic, fh, i], cols [f2, o_local, khw], bf16
    bf16_np = mybir.dt.np(mybir.dt.bfloat16)
    A = bank_weight.astype(np.float32).reshape(FH, FPT, OC, 128, IC, 128, KHW)
    #                     dims: (fh, f2, oc, o_local, ic, i, khw)
    core = A.transpose(2, 4, 0, 5, 1, 3, 6).reshape(OC * IC * FH * 128, TROW)
    bankT = np.zeros((OC * IC * FH * 128, TROW + CONST), dtype=np.float32)
    bankT[:, 0:TROW] = core
    bankT[0:128, TROW:TROW + 128] = np.eye(128, dtype=np.float32)
    bankT[0:128, TROW + 128] = 1.0
    bankT[0, TROW + 129:TROW + 257] = 1.0
    bankT = np.ascontiguousarray(bankT).astype(bf16_np)
    maps = []
    xpad = np.zeros((B, D, PH_, PW), dtype=np.float32)
    xpad[:, :, 1:1 + H, 1:1 + W] = x.astype(np.float32).reshape(B, D, H, W)
    xpad = xpad.astype(bf16_np)
    for b in range(B):
        maps.append({
            "x": np.ascontiguousarray(xpad[b].reshape(D, PH_ * PW)),
            "bank": bankT,
            "breq": np.ascontiguousarray(
                bank_request[b].astype(np.float32).reshape(1, F)),
            "sty": np.ascontiguousarray(style[b].astype(np.float32).reshape(1, D)),
        })
    return maps


def run(inputs, trace=False, **trace_kwargs):
    nc = _get_compiled()
    in_maps = _make_in_maps(inputs["x"], inputs["bank_request"],
                            inputs["style"], inputs["bank_weight"])
    # The first execution of a freshly compiled NEFF occasionally dies with
    # NRT_EXEC_UNIT_UNRECOVERABLE on this runtime; a plain retry succeeds.
    last_exc = None
    for _ in range(3):
        try:
            res = run_bass_kernel_spmd(nc, in_maps, core_ids=list(range(B)),
                                       trace=trace, **trace_kwargs)
            y = np.stack([res.results[b]["y"].reshape(D, H, W) for b in range(B)],
                         axis=0)
            return y, res
        except Exception as e:  # noqa: BLE001
            last_exc = e
    raise last_exc


def kernel(x, bank_request, style, bank_weight):
    y, _ = run({"x": np.asarray(x), "bank_request": np.asarray(bank_request),
                "style": np.asarray(style), "bank_weight": np.asarray(bank_weight)})
    return y


# revision 14
# speedup vs baseline: 1.0537x; 1.0471x over previous
"""BankModulatedConv Trainium2 kernel (v2: ic-split conv pipeline).

Problem (per sample b of B=8, one NeuronCore per sample):
  w = softmax(bank_request[b])                        # (16,)
  kern = sum_f w[f] * bank_weight[f]                  # (o, i, kh, kw) = (256, 256, 3, 3)
  kern *= (1 + style[b, i])                           # input-channel modulation
  kern *= rsqrt(sum_{i,kh,kw} kern^2 + 1e-8)          # per-o L2 demodulation
  y[b] = conv2d(x[b], kern, stride 1, SAME)           # (256, 64, 64)

v2 mapping (data-parallel over batch; all math on device):
  - The bank ships host-rearranged to 32 half-tiles
      [oc(2), ic(2), fh(8), i(128)] x [f2(2), o_local(128), khw(9)]  (bf16)
    issued in exact consumption order on the sync HWDGE queue with a
    shared 6-buffer pool tag, so arrival order tracks consumption and
    the stream self-paces: block (0,0) lands first, then x, then the
    remaining blocks back-to-back at aggregate HBM bandwidth.
  - Conv is split by input-channel half (ic): as soon as kern(oc0,ic0)
    is mixed (~1/4 of the bank stream), conv starts accumulating the
    ic0 taps of 4 spatial-tile PSUMs, leaving them open; the ic1 taps
    land when kern(oc0,ic1) is ready. This pulls ~17us of conv forward
    into what used to be DMA-starved mix time.
  - Style modulation is folded into the per-f mix weights
    (wsty[i,f] = softmax_w[f] * (1+style[i])), so mix PSUM->SBUF copies
    are plain copies and the DVE MAC chain needs no extra style pass.
  - Mix blocks (0,0), (0,1), (1,0) run on TensorE as 16 accumulated
    diag-weighted bf16 matmuls each; block (1,1) runs as a DVE
    scalar_tensor_tensor MAC chain so the PE can keep convolving.
  - demod: square + reduce-over-khw on DVE, cross-i reduction via a
    ones-vector matmul; rsqrt scale applied per output channel in the
    ScalarE conv-PSUM->y copy.
  - x ships host-pre-padded bf16; conv matmuls are bf16 (1 row/cycle,
    same as f32r, but half the DMA/SBUF traffic).
"""
import sys

if "/opt/trn_rl_repo" not in sys.path:
    sys.path.insert(0, "/opt/trn_rl_repo")

import numpy as np
import concourse.bacc as bacc
import concourse.mybir as mybir
import concourse.tile as tile
from concourse.alu_op_type import AluOpType
from concourse.bass_utils import run_bass_kernel_spmd

dt = mybir.dt
AF = mybir.ActivationFunctionType

B, F, D, KK, H, W = 8, 16, 256, 3, 64, 64
HW = H * W            # 4096
KHW = KK * KK         # 9
IC = D // 128         # 2 i-chunks
OC = D // 128         # 2 o-chunks
FH = 8                # f half-tiles per (oc, ic) block
FPT = 2               # f per half-tile
OCK = 128 * KHW       # 1152 free elems per (o_chunk, khw) group
TROW = FPT * OCK      # 2304 bf16 elems per DMA row
CONST = 257           # trailing constant columns on the first half-tile
PW = W + 2            # padded width 66
PH_ = H + 2           # padded height 66
NS = 8                # spatial tiles (8 rows each)
SROWS = H // NS       # 8 rows per spatial tile
SN = SROWS * W        # 512 = conv matmul moving size
G0 = (0, 1, 2, 3)     # first PSUM group of spatial tiles
G1 = (4, 5, 6, 7)

_COMPILED = None


def _build(num_devices=B):
    nc = bacc.Bacc("TRN2", target_bir_lowering=False, debug=False,
                   num_devices=num_devices)

    x_d = nc.dram_tensor("x", [D, PH_ * PW], dt.bfloat16, kind="ExternalInput").ap()
    bank_d = nc.dram_tensor("bank", [OC * IC * FH * 128, TROW + CONST], dt.bfloat16,
                            kind="ExternalInput").ap()
    breq_d = nc.dram_tensor("breq", [1, F], dt.float32, kind="ExternalInput").ap()
    sty_d = nc.dram_tensor("sty", [1, D], dt.float32, kind="ExternalInput").ap()
    y_d = nc.dram_tensor("y", [D, HW], dt.float32, kind="ExternalOutput").ap()

    f32, f32r, bf16 = dt.float32, dt.float32r, dt.bfloat16

    with tile.TileContext(nc) as tc:
        with (
            tc.tile_pool(name="setup", bufs=1) as setup,
            tc.tile_pool(name="xp", bufs=1) as xp,
            tc.tile_pool(name="bankp", bufs=6) as bankp,
            tc.tile_pool(name="kern", bufs=1) as kernp,
            tc.tile_pool(name="yout", bufs=4) as youtp,
            tc.tile_pool(name="mixps", bufs=1, space="PSUM") as mixps,
            tc.tile_pool(name="convps", bufs=4, space="PSUM") as convps,
            tc.tile_pool(name="normps", bufs=1, space="PSUM") as normps,
        ):
            # tiny control DMAs first so they land before the bank megabytes
            breq = setup.tile([1, F], dt.float32)
            nc.sync.dma_start(breq[:], breq_d[:])
            styrow = setup.tile([1, D], dt.float32)
            nc.sync.dma_start(styrow[:], sty_d[:])

            # ---------- bank + x DMAs, in exact consumption order ----------
            bts = {}

            def issue_block(oc, ic):
                for fh in range(FH):
                    wide = oc == 0 and ic == 0 and fh == 0
                    cols = TROW + CONST if wide else TROW
                    bt = bankp.tile([128, cols], bf16, tag="bank")
                    row0 = ((oc * IC + ic) * FH + fh) * 128
                    nc.sync.dma_start(bt[:], bank_d[row0:row0 + 128, 0:cols])
                    bts[(oc, ic, fh)] = bt

            issue_block(0, 0)
            # x ships as row-chunks so conv taps aren't gated on the whole-tile
            # DMA completing (chunk 0 covers G0's taps)
            XSPLIT = 36 * PW
            xpads = []
            for ic in range(IC):
                xpad = xp.tile([128, PH_ * PW], bf16, tag=f"xpad{ic}",
                               name=f"xpad{ic}")
                xpads.append(xpad)
            for ic, (lo, hi) in ((0, (0, XSPLIT)), (0, (XSPLIT, PH_ * PW)),
                                 (1, (0, XSPLIT)), (1, (XSPLIT, PH_ * PW))):
                nc.sync.dma_start(xpads[ic][:, lo:hi],
                                  x_d[ic * 128:(ic + 1) * 128, lo:hi])
            issue_block(0, 1)
            issue_block(1, 0)
            issue_block(1, 1)

            # ---------- setup: softmax weights, style, per-f diag tiles ----------
            bt0 = bts[(0, 0, 0)]
            ident = bt0[:, TROW:TROW + 128]            # I_128 (bf16-exact)
            onescol = bt0[:, TROW + 128:TROW + 129]
            onesrow_b = bt0[0:1, TROW + 129:TROW + 257]
            ones11_b = bt0[0:1, TROW + 129:TROW + 130]

            # softmax without the max-shift: inputs are O(1) so exp can't
            # overflow, and the shift is dead weight on the critical path
            ex = setup.tile([1, F], f32)
            nc.scalar.activation(ex[:], breq[:], AF.Exp, bias=0.0, scale=1.0)
            sm = setup.tile([1, 1], f32)
            nc.vector.reduce_sum(sm[:], ex[:], axis=mybir.AxisListType.X)
            rs = setup.tile([1, 1], f32)
            nc.vector.reciprocal(rs[:], sm[:])
            wrow = setup.tile([1, F], f32)
            nc.vector.tensor_scalar(out=wrow[:], in0=ex[:], scalar1=rs[:],
                                    scalar2=None, op0=AluOpType.mult)
            # broadcast w across partitions with a K=1 bf16 matmul
            wrow_b = setup.tile([1, F], bf16)
            with nc.allow_low_precision(reason="broadcast only"):
                nc.vector.tensor_copy(wrow_b[:], wrow[:])
            wbps = normps.tile([128, F], f32, tag="norm")
            nc.tensor.matmul(wbps[:], onesrow_b[:], wrow_b[:], start=True, stop=True)
            wbc = setup.tile([128, F], f32)
            nc.vector.tensor_copy(wbc[:], wbps[:])

            # (1 + style) spread to per-partition columns via K=1 matmuls
            sty1 = setup.tile([1, D], f32)
            nc.scalar.activation(sty1[:], styrow[:], AF.Copy, bias=1.0, scale=1.0)
            sty1b = setup.tile([1, D], bf16)
            with nc.allow_low_precision(reason="style factors; bf16 matches bank"):
                nc.vector.tensor_copy(sty1b[:], sty1[:])
            styps = normps.tile([128, IC], f32, tag="norm")
            stycols = []
            for ic in range(IC):
                nc.tensor.matmul(styps[:, ic:ic + 1],
                                 sty1b[0:1, ic * 128:(ic + 1) * 128],
                                 ones11_b, start=True, stop=True)
                sc = setup.tile([128, 1], f32, tag=f"sty{ic}")
                nc.scalar.activation(sc[:], styps[:, ic:ic + 1], AF.Copy,
                                     bias=0.0, scale=1.0)
                stycols.append(sc)

            # wsty[i, f] = w[f] * (1 + style[i]) -- style folded into mix weights
            wstys = []
            for ic in range(IC):
                ws = setup.tile([128, F], f32, tag=f"wsty{ic}")
                nc.vector.tensor_scalar(out=ws[:], in0=wbc[:],
                                        scalar1=stycols[ic][:],
                                        scalar2=None, op0=AluOpType.mult)
                wstys.append(ws)

            # copy the constants out of bt0 first: its pool slot must free as
            # soon as mix reads f0/f1, or the gated fh6/fh7/x dma triggers
            # (in-order sync queue) stall the x stream into conv's window
            idcp = setup.tile([128, 128], bf16)
            nc.vector.tensor_copy(idcp[:], ident)
            ones_r = setup.tile([128, 1], f32r)
            nc.vector.tensor_copy(ones_r[:], onescol)
            ones12 = setup.tile([1, 2], f32)
            nc.vector.memset(ones12[:], 1.0)

            # per-(ic, f) diagonal lhsT tiles diag(wsty), bf16 for the mix matmuls
            diags = {}
            with nc.allow_low_precision(reason="bf16 diag weights; mix accumulates f32"):
                for ic in range(IC):
                    for f in range(F):
                        dg = setup.tile([128, 128], bf16, tag=f"dg{ic}_{f}")
                        nc.vector.tensor_scalar(out=dg[:], in0=idcp[:],
                                                scalar1=wstys[ic][:, f:f + 1],
                                                scalar2=None, op0=AluOpType.mult)
                        diags[(ic, f)] = dg

            # ---------- mix / demod / norm helpers ----------
            SL = ((0, 512), (512, 1024), (1024, OCK))
            km = {}
            normcols = {}

            def mix_pe(oc, ic, copy_engine):
                kt = kernp.tile([128, OCK], bf16, tag=f"kern{oc}{ic}", name=f"kt{oc}{ic}")
                ps0 = mixps.tile([128, 512], f32, tag="mix0", name=f"m0{oc}{ic}")
                ps1 = mixps.tile([128, 512], f32, tag="mix1", name=f"m1{oc}{ic}")
                ps2 = mixps.tile([128, OCK - 1024], f32, tag="mix2", name=f"m2{oc}{ic}")
                pss = (ps0, ps1, ps2)
                for f in range(F):
                    bt = bts[(oc, ic, f // FPT)]
                    fo = (f % FPT) * OCK
                    for (lo, hi), ps in zip(SL, pss):
                        nc.tensor.matmul(ps[:], diags[(ic, f)][:],
                                         bt[:, fo + lo:fo + hi],
                                         start=(f == 0), stop=(f == F - 1))
                # copies chase the per-slice accumulation ends
                with nc.allow_low_precision(reason="kern stored bf16 for conv"):
                    for si, ((lo, hi), ps) in enumerate(zip(SL, pss)):
                        eng = copy_engine
                        if eng == "both":
                            eng = "scalar" if si == 1 else "vector"
                        if eng == "vector":
                            nc.vector.tensor_copy(kt[:, lo:hi], ps[:])
                        else:
                            nc.scalar.activation(kt[:, lo:hi], ps[:], AF.Copy,
                                                 bias=0.0, scale=1.0)
                km[(ic, oc)] = kt

            def mix_mac(oc, ic, mid_hook=None):
                # DVE scalar_tensor_tensor MAC chain; style already in wsty
                kt = kernp.tile([128, OCK], bf16, tag=f"kern{oc}{ic}", name=f"kt{oc}{ic}")
                acc0 = kernp.tile([128, OCK], f32, tag="macacc0", name="macacc0")
                acc1 = kernp.tile([128, OCK], f32, tag="macacc1", name="macacc1")
                accs = (acc0, acc1)
                ws = wstys[ic]
                with nc.allow_low_precision(reason="bf16 in, f32 acc, bf16 out"):
                    nc.vector.tensor_scalar(
                        out=accs[0][:], in0=bts[(oc, ic, 0)][:, 0:OCK],
                        scalar1=ws[:, 0:1], scalar2=None, op0=AluOpType.mult)
                    for f in range(1, F):
                        bt = bts[(oc, ic, f // FPT)]
                        fo = (f % FPT) * OCK
                        out = kt if f == F - 1 else accs[f % 2]
                        nc.vector.scalar_tensor_tensor(
                            out=out[:], in0=bt[:, fo:fo + OCK],
                            scalar=ws[:, f:f + 1], in1=accs[(f + 1) % 2][:],
                            op0=AluOpType.mult, op1=AluOpType.add)
                        if mid_hook is not None and f == mid_hook[0]:
                            mid_hook[1]()
                km[(ic, oc)] = kt

            def demod_dve(oc, ic):
                kt = km[(ic, oc)]
                scr = kernp.tile([128, OCK], f32r, tag="sqscratch", name=f"scr{oc}{ic}")
                nc.vector.tensor_mul(scr[:], kt[:], kt[:])
                redk = kernp.tile([128, 128], f32r, tag=f"redk{oc}{ic}",
                                  name=f"redk{oc}{ic}")
                with nc.allow_low_precision(reason="f32r is 4-byte"):
                    nc.vector.tensor_reduce(
                        redk[:], scr[:, :].rearrange("p (o r) -> p o r", r=KHW),
                        axis=mybir.AxisListType.X, op=AluOpType.add)
                return redk

            def demod_pe(npsum, redk, first, last):
                nc.tensor.matmul(npsum[:], ones_r[:], redk[:], start=first, stop=last)

            def norm_final(oc, npsum):
                nrow = setup.tile([1, 128], f32, tag=f"nrow{oc}", name=f"nrow{oc}")
                nc.vector.tensor_scalar_add(nrow[:], npsum[:], 1e-8)
                nsq = setup.tile([1, 128], f32, tag=f"nsq{oc}", name=f"nsq{oc}")
                nc.scalar.activation(nsq[:], nrow[:], AF.Sqrt, bias=0.0, scale=1.0)
                nrec = setup.tile([1, 128], f32, tag=f"nrec{oc}", name=f"nrec{oc}")
                nc.vector.reciprocal(nrec[:], nsq[:])
                ntr = normps.tile([128, 2], f32, tag="norm", name=f"ntr{oc}")
                nc.tensor.matmul(ntr[:], nrec[:], ones12[:], start=True, stop=True)
                ncol = setup.tile([128, 1], f32, tag=f"ncol{oc}", name=f"ncol{oc}")
                nc.scalar.activation(ncol[:], ntr[:, 0:1], AF.Copy, bias=0.0, scale=1.0)
                normcols[oc] = ncol

            # ---------- conv sweeps ----------
            cps = {}
            started = set()

            def conv_close(oc, s):
                yt = youtp.tile([128, SN], f32, tag="y", name=f"y{oc}{s}")
                nc.scalar.activation(yt[:], cps[(oc, s)][:], AF.Copy,
                                     bias=0.0, scale=normcols[oc][:])
                r0 = s * SROWS
                nc.scalar.dma_start(
                    y_d[oc * 128:(oc + 1) * 128, r0 * W:(r0 + SROWS) * W], yt[:])
                del cps[(oc, s)]

            def conv_mm(oc, s, ic, kh, kw, last):
                if (oc, s) not in cps:
                    cps[(oc, s)] = convps.tile([128, SN], f32, tag="conv",
                                               name=f"c{oc}{s}")
                r0 = s * SROWS
                xv = xpads[ic][:, :].rearrange("p (r c) -> p r c", c=PW)
                kv = km[(ic, oc)][:, :].rearrange("p (o r) -> p o r", r=KHW)
                nc.tensor.matmul(
                    cps[(oc, s)][:], kv[:, :, kh * KK + kw],
                    xv[:, r0 + kh:r0 + kh + SROWS, kw:kw + W],
                    start=(oc, s) not in started, stop=last)
                started.add((oc, s))

            def conv_tapmajor(oc, tiles, ic):
                # open sweep: tap-major (lhsT reused across tiles), no closes
                for kh in range(KK):
                    for kw in range(KK):
                        for s in tiles:
                            conv_mm(oc, s, ic, kh, kw, last=False)

            def conv_tilemajor(oc, tiles, ics):
                # closing sweep: tile-major so PSUM banks free progressively
                for s in tiles:
                    for ici, ic in enumerate(ics):
                        for kh in range(KK):
                            for kw in range(KK):
                                conv_mm(oc, s, ic, kh, kw,
                                        last=(ici == len(ics) - 1 and
                                              kh == KK - 1 and kw == KK - 1))
                    conv_close(oc, s)

            # ---------- schedule ----------
            # PE: mix00 -> G0(oc0) ic0 taps -> mix01 -> norm0 mms -> mix10
            #     -> G0(oc0) ic1 taps (close) -> G1(oc0) full (close)
            #     -> G0(oc1) ic0 taps -> norm1 mms -> G0(oc1) ic1 (close)
            #     -> G1(oc1) full (close)
            # DVE: diags -> kern copies/squares (0,*) -> norm0 tail
            #     -> MAC chain (1,1) with squares(1,0) spliced in -> norm1 tail
            mix_pe(0, 0, copy_engine="both")
            redk00 = demod_dve(0, 0)
            conv_tapmajor(0, G0, 0)
            npsum0 = normps.tile([1, 128], f32, tag="norm", name="npsum0")
            demod_pe(npsum0, redk00, True, False)
            mix_pe(0, 1, copy_engine="vector")
            redk01 = demod_dve(0, 1)

            # mix(1,0) is paced by its bank block's DMA arrival; fill the
            # pacing gap with tile s0's ic1 taps (kern(0,1) is ready by then)
            kt10 = kernp.tile([128, OCK], bf16, tag="kern10", name="kt10")
            ps010 = mixps.tile([128, 512], f32, tag="mix0", name="m010")
            ps110 = mixps.tile([128, 512], f32, tag="mix1", name="m110")
            ps210 = mixps.tile([128, OCK - 1024], f32, tag="mix2", name="m210")
            pss10 = (ps010, ps110, ps210)

            def mix10_mms(f_lo, f_hi):
                for f in range(f_lo, f_hi):
                    bt = bts[(1, 0, f // FPT)]
                    fo = (f % FPT) * OCK
                    for (lo, hi), ps in zip(SL, pss10):
                        nc.tensor.matmul(ps[:], diags[(0, f)][:],
                                         bt[:, fo + lo:fo + hi],
                                         start=(f == 0), stop=(f == F - 1))

            mix10_mms(0, 8)
            for kh in range(KK):
                for kw in range(KK):
                    conv_mm(0, 0, 1, kh, kw,
                            last=(kh == KK - 1 and kw == KK - 1))
            mix10_mms(8, F)
            with nc.allow_low_precision(reason="kern stored bf16 for conv"):
                for (lo, hi), ps in zip(SL, pss10):
                    nc.scalar.activation(kt10[:, lo:hi], ps[:], AF.Copy,
                                         bias=0.0, scale=1.0)
            km[(0, 1)] = kt10

            demod_pe(npsum0, redk01, False, True)
            norm_final(0, npsum0)

            # kern(1,1) mixed on DVE while PE convolves; demod(1,0)
            # squares are spliced into the chain once kern(1,0) lands
            redkbox = {}

            def sq10():
                redkbox["r10"] = demod_dve(1, 0)

            mix_mac(1, 1, mid_hook=(8, sq10))
            redk11 = demod_dve(1, 1)

            conv_close(0, 0)
            conv_tilemajor(0, (1, 2, 3), (1,))
            conv_tilemajor(0, G1, (0, 1))

            conv_tapmajor(1, G0, 0)
            npsum1 = normps.tile([1, 128], f32, tag="norm", name="npsum1")
            demod_pe(npsum1, redkbox["r10"], True, False)
            demod_pe(npsum1, redk11, False, True)
            norm_final(1, npsum1)
            conv_tilemajor(1, G0, (1,))
            conv_tilemajor(1, (4, 5, 6), (0, 1))

            # last spatial tile as two 4-row halves: the first half's y copy
            # and DMA overlap the second half's matmuls, trimming the tail
            for half in range(2):
                cp = convps.tile([128, SN // 2], f32, tag="conv",
                                 name=f"c17h{half}")
                r0 = 7 * SROWS + 4 * half
                for ici, ic in enumerate((0, 1)):
                    xv = xpads[ic][:, :].rearrange("p (r c) -> p r c", c=PW)
                    kv = km[(ic, 1)][:, :].rearrange("p (o r) -> p o r", r=KHW)
                    for kh in range(KK):
                        for kw in range(KK):
                            nc.tensor.matmul(
                                cp[:], kv[:, :, kh * KK + kw],
                                xv[:, r0 + kh:r0 + kh + 4, kw:kw + W],
                                start=(ici == 0 and kh == 0 and kw == 0),
                                stop=(ici == 1 and kh == KK - 1 and kw == KK - 1))
                yt = youtp.tile([128, SN // 2], f32, tag="y", name=f"y17h{half}")
                nc.scalar.activation(yt[:], cp[:], AF.Copy,
                                     bias=0.0, scale=normcols[1][:])
                nc.scalar.dma_start(
                    y_d[128:256, r0 * W:(r0 + 4) * W], yt[:])

    nc.compile()
    return nc


def _get_compiled():
    global _COMPILED
    if _COMPILED is None:
        _COMPILED = _build()
    return _COMPILED


def _make_in_maps(x, bank_request, style, bank_weight):
    # bank: (F, O, I, KH, KW) -> rows [oc, ic, fh, i], cols [f2, o_local, khw], bf16
    bf16_np = mybir.dt.np(mybir.dt.bfloat16)
    A = bank_weight.astype(np.float32).reshape(FH, FPT, OC, 128, IC, 128, KHW)
    #                     dims: (fh, f2, oc, o_local, ic, i, khw)
    core = A.transpose(2, 4, 0, 5, 1, 3, 6).reshape(OC * IC * FH * 128, TROW)
    bankT = np.zeros((OC * IC * FH * 128, TROW + CONST), dtype=np.float32)
    bankT[:, 0:TROW] = core
    bankT[0:128, TROW:TROW + 128] = np.eye(128, dtype=np.float32)
    bankT[0:128, TROW + 128] = 1.0
    bankT[0, TROW + 129:TROW + 257] = 1.0
    bankT = np.ascontiguousarray(bankT).astype(bf16_np)
    maps = []
    xpad = np.zeros((B, D, PH_, PW), dtype=np.float32)
    xpad[:, :, 1:1 + H, 1:1 + W] = x.astype(np.float32).reshape(B, D, H, W)
    xpad = xpad.astype(bf16_np)
    for b in range(B):
        maps.append({
            "x": np.ascontiguousarray(xpad[b].reshape(D, PH_ * PW)),
            "bank": bankT,
            "breq": np.ascontiguousarray(
                bank_request[b].astype(np.float32).reshape(1, F)),
            "sty": np.ascontiguousarray(style[b].astype(np.float32).reshape(1, D)),
        })
    return maps


def run(inputs, trace=False, **trace_kwargs):
    nc = _get_compiled()
    in_maps = _make_in_maps(inputs["x"], inputs["bank_request"],
                            inputs["style"], inputs["bank_weight"])
    # The first execution of a freshly compiled NEFF occasionally dies with
    # NRT_EXEC_UNIT_UNRECOVERABLE on this runtime; a plain retry succeeds.
    last_exc = None
    for _ in range(3):
        try:
            res = run_bass_kernel_spmd(nc, in_maps, core_ids=list(range(B)),
                                       trace=trace, **trace_kwargs)
            y = np.stack([res.results[b]["y"].reshape(D, H, W) for b in range(B)],
                         axis=0)
            return y, res
        except Exception as e:  # noqa: BLE001
            last_exc = e
    raise last_exc


def kernel(x, bank_request, style, bank_weight):
    y, _ = run({"x": np.asarray(x), "bank_request": np.asarray(bank_request),
                "style": np.asarray(style), "bank_weight": np.asarray(bank_weight)})
    return y
